# revision 1
# baseline (speedup 1.0000x reference)
"""Cross-attention (causal) Trainium2 kernel, 8-core SPMD.

Sharding: core c -> batch c//2, decoder-row half c%2.
Half 0 owns 128-row q-blocks {0,3,4,7}, half 1 owns {1,2,5,6} of T_dec=1024.
This balances causal-attention work exactly (18 key-block units each) with
zero collectives: output rows are disjoint, host reassembles.

Per-core kernel (channel-major activations, fp32r matmuls):
  XdT/XeT  <- PE-transpose of inputs
  QT=Wq@XdT+bq, KT=Wk@XeT+bk (channel-major), V=Xe@WvT+bv (token-major,
  augmented with a ones column per head so attn row-sums come free)
  per head h, key-block j: S^T = KT_h^T-slice @ QT_h (keys x q), p=exp(S/8),
  mask-multiply one 128-col window (host-supplied causal masks),
  AV psum accumulates [V_h|1]^T @ p -> rows 0..63 = y^T, row 64 = l
  ynorm^T = y^T * bcast(1/l);  out = ynorm^T.T @ WpT + bp (token-major)
"""

import numpy as np

P = 128
DE = 1024          # emb dim
Q = 512            # q rows per core
H = 16
HD = 64
ET = DE // P       # 8 e-tiles
# active q-cols per key-block; j=6,7 padded 128->256 (fp32r needs N>=256 for
# 1 cyc/row; the extra 128 always-invalid cols are zeroed before AV)
N_J = [512, 512, 384, 384, 256, 256, 256, 256]
QB = ([0, 3, 4, 7], [1, 2, 5, 6])                # q-block assignment per half

_NC_CACHE = {}


def _build_nc():
    import concourse.tile as tile
    from concourse import bacc, mybir
    from concourse.masks import make_identity

    F32 = mybir.dt.float32
    F32R = mybir.dt.float32r
    AF = mybir.ActivationFunctionType

    nc = bacc.Bacc("TRN2", target_bir_lowering=False, debug=False)

    x_enc = nc.dram_tensor("x_enc", [DE, DE], F32, kind="ExternalInput").ap()
    x_dec = nc.dram_tensor("x_dec", [Q, DE], F32, kind="ExternalInput").ap()
    Wq = nc.dram_tensor("Wq", [DE, DE], F32, kind="ExternalInput").ap()
    Wk = nc.dram_tensor("Wk", [DE, DE], F32, kind="ExternalInput").ap()
    Wv = nc.dram_tensor("Wv", [DE, DE], F32, kind="ExternalInput").ap()
    Wp = nc.dram_tensor("Wp", [DE, DE], F32, kind="ExternalInput").ap()
    bq = nc.dram_tensor("bq", [DE], F32, kind="ExternalInput").ap()
    bk = nc.dram_tensor("bk", [DE], F32, kind="ExternalInput").ap()
    bv = nc.dram_tensor("bv", [DE], F32, kind="ExternalInput").ap()
    bp = nc.dram_tensor("bp", [DE], F32, kind="ExternalInput").ap()
    masks = nc.dram_tensor("masks", [8, P, P], F32, kind="ExternalInput").ap()
    out = nc.dram_tensor("out", [Q, DE], F32, kind="ExternalOutput").ap()

    with tile.TileContext(nc) as tc:
        with tc.tile_pool(name="persist", bufs=1) as pp, \
             tc.tile_pool(name="consts", bufs=1) as cp:
            ident_f = cp.tile([P, P], F32)
            make_identity(nc, ident_f)
            # fp32r identity -> single-pass PE transposes (1.5 vs 2 cyc/row);
            # exact: transpose only multiplies by 1. DMA sources are bitcast
            # to fp32r so the BIR verifier sees fp32r producers end-to-end.
            ident = cp.tile([P, P], F32R)
            nc.vector.tensor_copy(ident[:], ident_f[:])

            def pe_transpose(out_ps, in_ap):
                nc.tensor.transpose(out_ps, in_ap, ident[:])
            ones_f = cp.tile([1, P], F32)
            nc.vector.memset(ones_f, 1.0)
            ones_r = cp.tile([1, P], F32R)
            nc.vector.tensor_copy(ones_r[:], ones_f[:])
            ones16 = cp.tile([P, H], F32)
            nc.vector.memset(ones16, 1.0)

            # biases: [p, t] = b[128t + p]
            bq_sb = cp.tile([P, ET], F32)
            nc.gpsimd.dma_start(out=bq_sb, in_=bq.rearrange("(t p) -> p t", p=P))
            bk_sb = cp.tile([P, ET], F32)
            nc.gpsimd.dma_start(out=bk_sb, in_=bk.rearrange("(t p) -> p t", p=P))
            bv_f = cp.tile([1, DE], F32)
            nc.gpsimd.dma_start(out=bv_f, in_=bv[None, :])
            bv_r = cp.tile([1, DE], F32R)
            nc.vector.tensor_copy(bv_r[:], bv_f[:])
            bp_f = cp.tile([1, DE], F32)
            nc.gpsimd.dma_start(out=bp_f, in_=bp[None, :])
            bp_r = cp.tile([1, DE], F32R)
            nc.vector.tensor_copy(bp_r[:], bp_f[:])

            masks_sb = cp.tile([P, 8, P], F32)
            nc.sync.dma_start(out=masks_sb, in_=masks.rearrange("j r c -> r j c"))

            # persistent activation tensors
            QT = [pp.tile([P, Q], F32R, name=f"QT{i}") for i in range(ET)]
            KT = [pp.tile([P, DE], F32R, name=f"KT{i}") for i in range(ET)]
            VA = [pp.tile([P, H * (HD + 1)], F32R, name=f"VA{i}") for i in range(ET)]
            YT = [pp.tile([P, Q], F32R, name=f"YT{i}") for i in range(ET)]

            # ---------------- phase 1: transpose x_dec / x_enc ----------
            XDT = None
            XET = None
            with tc.tile_pool(name="xt", bufs=1) as xtp:
                XDT = [xtp.tile([P, Q], F32R, name=f"XDT{i}") for i in range(ET)]
                XET = [xtp.tile([P, DE], F32R, name=f"XET{i}") for i in range(ET)]
                with tc.tile_pool(name="ps1", bufs=3, space="PSUM") as ps1, \
                     tc.tile_pool(name="nat", bufs=5) as natp:
                    xd_nat = []
                    for t in range(4):
                        xt_ = natp.tile([P, DE], F32R, name=f"xdn{t}", tag="xdn")
                        nc.sync.dma_start(
                            out=xt_,
                            in_=x_dec[t * P:(t + 1) * P, :].bitcast(F32R))
                        xd_nat.append(xt_)
                    for e in range(ET):
                        pst = ps1.tile([P, Q], F32R, tag="ps1")
                        for t in range(4):
                            pe_transpose(
                                pst[:, t * P:(t + 1) * P],
                                xd_nat[t][:, e * P:(e + 1) * P])
                        eng = nc.scalar if e % 2 == 0 else nc.vector
                        if e % 2 == 0:
                            nc.scalar.copy(XDT[e][:], pst[:])
                        else:
                            nc.vector.tensor_copy(XDT[e][:], pst[:])
                with tc.tile_pool(name="ps1b", bufs=3, space="PSUM") as ps1, \
                     tc.tile_pool(name="natb", bufs=5) as natp:
                    for half in range(2):
                        xe_nat = []
                        for t in range(4):
                            xt_ = natp.tile([P, DE], F32R, name=f"xen{t}",
                                            tag="xen")
                            nc.sync.dma_start(
                                out=xt_,
                                in_=x_enc[(4 * half + t) * P:
                                          (4 * half + t + 1) * P, :]
                                .bitcast(F32R))
                            xe_nat.append(xt_)
                        for e in range(ET):
                            pst = ps1.tile([P, Q], F32R, tag="ps1b")
                            for t in range(4):
                                pe_transpose(
                                    pst[:, t * P:(t + 1) * P],
                                    xe_nat[t][:, e * P:(e + 1) * P])
                            dst = XET[e][:, half * Q:(half + 1) * Q]
                            if (e + half) % 2 == 0:
                                nc.scalar.copy(dst, pst[:])
                            else:
                                nc.vector.tensor_copy(dst, pst[:])

                # ------------- phase 2: projections ----------------------
                with tc.tile_pool(name="ps2t", bufs=3, space="PSUM") as ps2t, \
                     tc.tile_pool(name="ps2", bufs=3, space="PSUM") as ps2, \
                     tc.tile_pool(name="wblk", bufs=8) as wblkp, \
                     tc.tile_pool(name="wt", bufs=8) as wtp:

                    def wT_panel(W, e):
                        """Build W^T panel [128(e), 1024(dout)] for e-tile e."""
                        wte = wtp.tile([P, DE], F32R, name=f"wT{e}", tag="wt")
                        for half in range(2):
                            pst = ps2t.tile([P, Q], F32R, tag="ps2t")
                            for d in range(4):
                                dd = 4 * half + d
                                blk = wblkp.tile([P, P], F32R, name="wb", tag="wb")
                                nc.sync.dma_start(
                                    out=blk,
                                    in_=W[dd * P:(dd + 1) * P,
                                          e * P:(e + 1) * P].bitcast(F32R))
                                pe_transpose(
                                    pst[:, d * P:(d + 1) * P], blk[:])
                            dst = wte[:, half * Q:(half + 1) * Q]
                            if half % 2 == 0:
                                nc.scalar.copy(dst, pst[:])
                            else:
                                nc.vector.tensor_copy(dst, pst[:])
                        return wte

                    # --- Q projection: QT[d] = Wq @ XdT + bq
                    wqt = [wT_panel(Wq, e) for e in range(ET)]
                    for d in range(ET):
                        psq = ps2.tile([P, Q], F32, tag="ps2")
                        for e in range(ET):
                            nc.tensor.matmul(
                                psq[:], wqt[e][:, d * P:(d + 1) * P], XDT[e][:],
                                start=(e == 0), stop=(e == ET - 1))
                        nc.scalar.activation(QT[d][:], psq[:], AF.Identity,
                                             bias=bq_sb[:, d:d + 1])
                    # --- K projection: KT[d] = Wk @ XeT + bk
                    wkt = [wT_panel(Wk, e) for e in range(ET)]
                    for d in range(ET):
                        for ch in range(2):
                            psk = ps2.tile([P, Q], F32, tag="ps2")
                            for e in range(ET):
                                nc.tensor.matmul(
                                    psk[:], wkt[e][:, d * P:(d + 1) * P],
                                    XET[e][:, ch * Q:(ch + 1) * Q],
                                    start=(e == 0), stop=(e == ET - 1))
                            nc.scalar.activation(
                                KT[d][:, ch * Q:(ch + 1) * Q], psk[:],
                                AF.Identity, bias=bk_sb[:, d:d + 1])
                    # --- V projection (token-major, augmented)
                    wvt = [wT_panel(Wv, e) for e in range(ET)]
                    for kt in range(ET):
                        for ch in range(2):
                            psv = ps2.tile([P, Q], F32, tag="ps2")
                            for e in range(ET):
                                nc.tensor.matmul(
                                    psv[:], XET[e][:, kt * P:(kt + 1) * P],
                                    wvt[e][:, ch * Q:(ch + 1) * Q],
                                    start=(e == 0), stop=False)
                            nc.tensor.matmul(
                                psv[:], ones_r[:], bv_r[:, ch * Q:(ch + 1) * Q],
                                start=False, stop=True)
                            # scatter 8 heads into VA (65-col stride per head)
                            hbase = 8 * ch
                            dst = VA[kt][:, hbase * (HD + 1):(hbase + 8) * (HD + 1)]
                            dst = dst.rearrange("p (h x) -> p h x", h=8)[:, :, :HD]
                            src = psv.rearrange("p (h x) -> p h x", h=8)
                            nc.vector.tensor_copy(dst, src)
                        # ones column per head (col 64 of each 65-block)
                        onesdst = VA[kt].rearrange(
                            "p (h x) -> p h x", x=HD + 1)[:, :, HD:HD + 1]
                        nc.vector.tensor_copy(
                            onesdst, ones16.rearrange("p (h x) -> p h x", x=1))

            # ------- phase 3 + 4: attention, with Wp^T hoisted early -----
            with tc.tile_pool(name="ps4t", bufs=2, space="PSUM") as ps4t, \
                 tc.tile_pool(name="wblk4", bufs=8) as wblkp, \
                 tc.tile_pool(name="wt4", bufs=8) as wtp:

                def wT_panel4(W, e):
                    wte = wtp.tile([P, DE], F32R, name=f"wpT{e}", tag="wt4")
                    for half in range(2):
                        pst = ps4t.tile([P, Q], F32R, tag="ps4t")
                        for d in range(4):
                            dd = 4 * half + d
                            blk = wblkp.tile([P, P], F32R, name="wb4", tag="wb4")
                            nc.sync.dma_start(
                                out=blk,
                                in_=W[dd * P:(dd + 1) * P,
                                      e * P:(e + 1) * P].bitcast(F32R))
                            pe_transpose(
                                pst[:, d * P:(d + 1) * P], blk[:])
                        dst = wte[:, half * Q:(half + 1) * Q]
                        if half % 2 == 0:
                            nc.scalar.copy(dst, pst[:])
                        else:
                            nc.vector.tensor_copy(dst, pst[:])
                    return wte

                wpt = [wT_panel4(Wp, e) for e in range(ET)]

                with tc.tile_pool(name="ps3s", bufs=3, space="PSUM") as ps3s, \
                     tc.tile_pool(name="ps3a", bufs=3, space="PSUM") as ps3a, \
                     tc.tile_pool(name="pt", bufs=6) as ptp, \
                     tc.tile_pool(name="sm", bufs=4) as smp:
                    for h in range(H):
                        ht, off = h // 2, HD * (h % 2)
                        av = ps3a.tile([HD + 1, Q], F32, tag="av")
                        for j in range(8):
                            nj = N_J[j]
                            cs = Q - nj
                            st = ps3s.tile([P, Q], F32, tag="st")
                            nc.tensor.matmul(
                                st[:, :nj],
                                KT[ht][off:off + HD, j * P:(j + 1) * P],
                                QT[ht][off:off + HD, cs:],
                                start=True, stop=True)
                            pt = ptp.tile([P, Q], F32R, tag="pt")
                            nc.scalar.activation(pt[:, :nj], st[:, :nj], AF.Exp,
                                                 scale=0.125)
                            moff = P * (j // 2) - cs
                            if moff > 0:
                                nc.scalar.mul(pt[:, 0:moff], pt[:, 0:moff], 0.0)
                            nc.vector.tensor_mul(pt[:, moff:moff + P],
                                                 pt[:, moff:moff + P],
                                                 masks_sb[:, j, :])
                            nc.tensor.matmul(
                                av[:, cs:],
                                VA[j][:, h * (HD + 1):(h + 1) * (HD + 1)],
                                pt[:, :nj], start=(j == 0), stop=(j == 7))
                        # deferred softmax normalization: broadcast l, then
                        # reciprocal on 64 partitions (not 1 — DVE lane use)
                        lrow = smp.tile([1, Q], F32, tag="lrow")
                        nc.scalar.copy(lrow[:], av[HD:HD + 1, :])
                        lb = smp.tile([HD, Q], F32, tag="lb")
                        nc.gpsimd.partition_broadcast(lb[:], lrow[:])
                        rcp = smp.tile([HD, Q], F32, tag="rcp")
                        nc.vector.reciprocal_approx_fast(out=rcp[:], in_=lb[:])
                        nc.vector.tensor_mul(YT[ht][off:off + HD, :],
                                             av[:HD, :], rcp[:])

            # ---------------- phase 4: output projection -----------------
                with tc.tile_pool(name="ps4", bufs=3, space="PSUM") as ps4, \
                     tc.tile_pool(name="osb", bufs=3) as osbp:
                    for m in range(4):
                        osb = osbp.tile([P, DE], F32, tag="osb")
                        for ch in range(2):
                            pso = ps4.tile([P, Q], F32, tag="ps4")
                            for a in range(ET):
                                nc.tensor.matmul(
                                    pso[:], YT[a][:, m * P:(m + 1) * P],
                                    wpt[a][:, ch * Q:(ch + 1) * Q],
                                    start=(a == 0), stop=False)
                            nc.tensor.matmul(
                                pso[:], ones_r[:], bp_r[:, ch * Q:(ch + 1) * Q],
                                start=False, stop=True)
                            nc.scalar.copy(osb[:, ch * Q:(ch + 1) * Q], pso[:])
                        nc.sync.dma_start(out=out[m * P:(m + 1) * P, :],
                                          in_=osb[:])

    nc.compile()
    return nc


def get_nc():
    if "nc" not in _NC_CACHE:
        _NC_CACHE["nc"] = _build_nc()
    return _NC_CACHE["nc"]


def make_masks(qblocks):
    m = np.zeros((8, P, P), dtype=np.float32)
    for j in range(8):
        p = j // 2
        gq = P * qblocks[p] + np.arange(P)[None, :]
        gk = P * j + np.arange(P)[:, None]
        m[j] = (gk <= gq).astype(np.float32)
    return m


def shard_inputs(x_encoder, x_decoder, Wq, bq, Wk, bk, Wv, bv, Wp, bp):
    c = np.ascontiguousarray
    in_maps = []
    for core in range(8):
        b, half = core // 2, core % 2
        qb = QB[half]
        xd = np.concatenate([x_decoder[b, P * t:P * (t + 1)] for t in qb], 0)
        in_maps.append({
            "x_enc": c(x_encoder[b]).astype(np.float32),
            "x_dec": c(xd).astype(np.float32),
            "Wq": c(Wq).astype(np.float32), "bq": c(bq).astype(np.float32),
            "Wk": c(Wk).astype(np.float32), "bk": c(bk).astype(np.float32),
            "Wv": c(Wv).astype(np.float32), "bv": c(bv).astype(np.float32),
            "Wp": c(Wp).astype(np.float32), "bp": c(bp).astype(np.float32),
            "masks": make_masks(qb),
        })
    return in_maps


def assemble(results, B=4, T=1024):
    out = np.zeros((B, T, DE), dtype=np.float32)
    for core in range(8):
        b, half = core // 2, core % 2
        for p, t in enumerate(QB[half]):
            out[b, P * t:P * (t + 1)] = results[core]["out"][P * p:P * (p + 1)]
    return out


def kernel(**inputs):
    from concourse.bass_utils import run_bass_kernel_spmd
    nc = get_nc()
    in_maps = shard_inputs(**{k: np.asarray(v) for k, v in inputs.items()})
    res = run_bass_kernel_spmd(nc, in_maps, core_ids=list(range(8)))
    return assemble(res.results)


if __name__ == "__main__":
    nc = get_nc()
    print("built + compiled ok")



# revision 4
# speedup vs baseline: 1.6736x; 1.6736x over previous
"""Cross-attention (causal) Trainium2 kernel, 8-core SPMD, bf16 compute.

Sharding: core c -> batch c//2, decoder-row half c%2.
Half 0 owns 128-row q-blocks {0,3,4,7}, half 1 owns {1,2,5,6} of T_dec=1024.
This balances causal-attention work (18 key-block units each) with zero
collectives: output rows are disjoint, host reassembles.

Host-side layout prep (shard_inputs): X and W are pre-transposed to
emb-major and cast to bf16 on the host, so the device kernel runs zero
PE transposes and half the DMA bytes.  All matmuls are bf16 with fp32
PSUM accumulation (measured end-to-end rel err ~2.4e-3, gate 2e-2).

Per-core kernel:
  QT = (WqT panel).T @ XdT + bq   (channel-major, d-outer / e-accumulate)
  KT likewise over the full encoder; V token-major, augmented with a
  per-head ones column so attention row-sums (softmax denominators)
  come free as row 64 of the AV psum.
  Attention runs per head-PAIR: heads 2t/2t+1 live on partitions 0-63 /
  64-127, so their S^T matmuls occupy disjoint PE row-groups and execute
  concurrently (auto tile_position from base partitions).  V-projection
  key-block kt is interleaved into pair 0's j-loop to hide exp latency;
  later pairs software-pipeline S_{j+1} between S_j and AV_j.
  N_J trims each key-block's matmul to the causally active q-column
  suffix; the tri/zero/ones mask window is always the first 128 columns
  (host-supplied per-core masks, same make_masks as before).
  ynorm^T = av[:64] * bcast(1/av[64]); out = ynorm^T.T @ WpT + bp.
"""

import numpy as np

P = 128
DE = 1024          # emb dim
Q = 512            # q rows per core
H = 16
HD = 64
ET = DE // P       # 8 e-tiles
# active q-cols per key-block j (shared max over both halves); the causal
# suffix starts at column 128*(j//2), so the mask window is always cols
# [0:128) of the active slice.
N_J = [512, 512, 384, 384, 256, 256, 128, 128]
QB = ([0, 3, 4, 7], [1, 2, 5, 6])                # q-block assignment per half

_NC_CACHE = {}


def _build_nc():
    import concourse.tile as tile
    from concourse import bacc, mybir

    F32 = mybir.dt.float32
    BF16 = mybir.dt.bfloat16
    AF = mybir.ActivationFunctionType

    nc = bacc.Bacc("TRN2", target_bir_lowering=False, debug=False)

    xdT = nc.dram_tensor("xdT", [DE, Q], BF16, kind="ExternalInput").ap()
    xeT = nc.dram_tensor("xeT", [DE, DE], BF16, kind="ExternalInput").ap()
    wqT = nc.dram_tensor("wqT", [DE, DE], BF16, kind="ExternalInput").ap()
    wkT = nc.dram_tensor("wkT", [DE, DE], BF16, kind="ExternalInput").ap()
    wvT = nc.dram_tensor("wvT", [DE, DE], BF16, kind="ExternalInput").ap()
    wpT = nc.dram_tensor("wpT", [DE, DE], BF16, kind="ExternalInput").ap()
    bq = nc.dram_tensor("bq", [DE], F32, kind="ExternalInput").ap()
    bk = nc.dram_tensor("bk", [DE], F32, kind="ExternalInput").ap()
    bvb = nc.dram_tensor("bvb", [DE], BF16, kind="ExternalInput").ap()
    bpb = nc.dram_tensor("bpb", [DE], BF16, kind="ExternalInput").ap()
    masks = nc.dram_tensor("masks", [8, P, P], BF16, kind="ExternalInput").ap()
    out = nc.dram_tensor("out", [Q, DE], F32, kind="ExternalOutput").ap()

    with tile.TileContext(nc) as tc:
        with tc.tile_pool(name="persist", bufs=1) as pp, \
             tc.tile_pool(name="consts", bufs=1) as cp:
            ones1 = cp.tile([1, P], BF16)
            nc.vector.memset(ones1, 1.0)
            ones16 = cp.tile([P, H], BF16)
            nc.vector.memset(ones16, 1.0)

            # biases: [p, t] = b[128t + p]
            bq_sb = cp.tile([P, ET], F32)
            nc.gpsimd.dma_start(out=bq_sb, in_=bq.rearrange("(t p) -> p t", p=P))
            bk_sb = cp.tile([P, ET], F32)
            nc.gpsimd.dma_start(out=bk_sb, in_=bk.rearrange("(t p) -> p t", p=P))
            bv_row = cp.tile([1, DE], BF16)
            nc.gpsimd.dma_start(out=bv_row, in_=bvb[None, :])
            bp_row = cp.tile([1, DE], BF16)
            nc.gpsimd.dma_start(out=bp_row, in_=bpb[None, :])
            masks_sb = cp.tile([P, 8, P], BF16)
            nc.sync.dma_start(out=masks_sb, in_=masks.rearrange("j r c -> r j c"))

            # persistent activations + weight panels (all bf16, emb-major)
            XdT = [pp.tile([P, Q], BF16, name=f"XdT{i}") for i in range(ET)]
            XeT = [pp.tile([P, DE], BF16, name=f"XeT{i}") for i in range(ET)]
            WQ = [pp.tile([P, DE], BF16, name=f"WQ{i}") for i in range(ET)]
            WK = [pp.tile([P, DE], BF16, name=f"WK{i}") for i in range(ET)]
            WV = [pp.tile([P, DE], BF16, name=f"WV{i}") for i in range(ET)]
            WP = [pp.tile([P, DE], BF16, name=f"WP{i}") for i in range(ET)]
            QT = [pp.tile([P, Q], BF16, name=f"QT{i}") for i in range(ET)]
            KT = [pp.tile([P, DE], BF16, name=f"KT{i}") for i in range(ET)]
            VA = [pp.tile([P, H * (HD + 1)], BF16, name=f"VA{i}")
                  for i in range(ET)]
            YT = [pp.tile([P, Q], BF16, name=f"YT{i}") for i in range(ET)]

            # DMA prefetch, in consumption order: (XdT, WQ) -> (XeT, WK)
            # -> WV -> WP.  Tile spreads these over the DMA rings; issue
            # order sets the streaming priority.
            for e in range(ET):
                nc.sync.dma_start(out=XdT[e], in_=xdT[e * P:(e + 1) * P, :])
                nc.sync.dma_start(out=WQ[e], in_=wqT[e * P:(e + 1) * P, :])
            for e in range(ET):
                nc.sync.dma_start(out=XeT[e], in_=xeT[e * P:(e + 1) * P, :])
                nc.sync.dma_start(out=WK[e], in_=wkT[e * P:(e + 1) * P, :])
            for e in range(ET):
                nc.sync.dma_start(out=WV[e], in_=wvT[e * P:(e + 1) * P, :])
            for e in range(ET):
                nc.sync.dma_start(out=WP[e], in_=wpT[e * P:(e + 1) * P, :])

            with tc.tile_pool(name="ps", bufs=2, space="PSUM") as ps, \
                 tc.tile_pool(name="st", bufs=3, space="PSUM") as stp, \
                 tc.tile_pool(name="av", bufs=3, space="PSUM") as avp, \
                 tc.tile_pool(name="pt", bufs=4) as ptp, \
                 tc.tile_pool(name="sm", bufs=4) as smp, \
                 tc.tile_pool(name="osb", bufs=2) as osbp:

                # ---- Q projection: QT[d] = WQ[:,d].T @ XdT + bq ---------
                for d in range(ET):
                    psq = ps.tile([P, Q], F32, tag="ps")
                    for e in range(ET):
                        nc.tensor.matmul(
                            psq[:], WQ[e][:, d * P:(d + 1) * P], XdT[e][:],
                            start=(e == 0), stop=(e == ET - 1))
                    nc.scalar.activation(QT[d][:], psq[:], AF.Identity,
                                         bias=bq_sb[:, d:d + 1])

                # ---- K projection (full encoder, 2 column chunks) -------
                for d in range(ET):
                    for ch in range(2):
                        psk = ps.tile([P, Q], F32, tag="ps")
                        for e in range(ET):
                            nc.tensor.matmul(
                                psk[:], WK[e][:, d * P:(d + 1) * P],
                                XeT[e][:, ch * Q:(ch + 1) * Q],
                                start=(e == 0), stop=(e == ET - 1))
                        nc.scalar.activation(
                            KT[d][:, ch * Q:(ch + 1) * Q], psk[:],
                            AF.Identity, bias=bk_sb[:, d:d + 1])

                # ---- V projection for one key block (token-major, the
                # per-head ones column makes attention row-sums free) -----
                def emit_v(kt):
                    for ch in range(2):
                        psv = ps.tile([P, Q], F32, tag="ps")
                        for e in range(ET):
                            nc.tensor.matmul(
                                psv[:], XeT[e][:, kt * P:(kt + 1) * P],
                                WV[e][:, ch * Q:(ch + 1) * Q],
                                start=(e == 0), stop=False)
                        nc.tensor.matmul(
                            psv[:], ones1[:], bv_row[:, ch * Q:(ch + 1) * Q],
                            start=False, stop=True)
                        hbase = 8 * ch
                        dst = VA[kt][:, hbase * (HD + 1):(hbase + 8) * (HD + 1)]
                        dst = dst.rearrange("p (h x) -> p h x", h=8)[:, :, :HD]
                        src = psv.rearrange("p (h x) -> p h x", h=8)
                        nc.vector.tensor_copy(dst, src)
                    onesdst = VA[kt].rearrange(
                        "p (h x) -> p h x", x=HD + 1)[:, :, HD:HD + 1]
                    nc.vector.tensor_copy(
                        onesdst, ones16.rearrange("p (h x) -> p h x", x=1))

                # ---- attention, one head-pair at a time -----------------
                def emit_s(ht, j):
                    nj = N_J[j]
                    cs = P * (j // 2)
                    pts = []
                    for off in (0, HD):  # head 2ht on rows 0-63, 2ht+1 on 64-127
                        st = stp.tile([P, Q], F32, tag="st")
                        nc.tensor.matmul(
                            st[:, :nj],
                            KT[ht][off:off + HD, j * P:(j + 1) * P],
                            QT[ht][off:off + HD, cs:cs + nj],
                            start=True, stop=True)
                        pt = ptp.tile([P, Q], BF16, tag="pt")
                        nc.scalar.activation(pt[:, :nj], st[:, :nj], AF.Exp,
                                             scale=0.125)
                        nc.vector.tensor_mul(pt[:, :P], pt[:, :P],
                                             masks_sb[:, j, :])
                        pts.append(pt)
                    return pts

                def emit_av(ht, j, pts, avs):
                    nj = N_J[j]
                    cs = P * (j // 2)
                    for i, h in enumerate((2 * ht, 2 * ht + 1)):
                        nc.tensor.matmul(
                            avs[i][:, cs:cs + nj],
                            VA[j][:, h * (HD + 1):(h + 1) * (HD + 1)],
                            pts[i][:, :nj], start=(j == 0), stop=(j == 7))

                for ht in range(ET):
                    avs = [avp.tile([HD + 1, Q], F32, name=f"av{ht}_{i}",
                                    tag="av")
                           for i in range(2)]
                    pend = emit_s(ht, 0)
                    for j in range(8):
                        if ht == 0:
                            emit_v(j)  # hides pair-0 exp latency on the PE
                        nxt = emit_s(ht, j + 1) if j < 7 else None
                        emit_av(ht, j, pend, avs)
                        pend = nxt
                    # deferred softmax normalization: 1/l on the row, then
                    # partition-broadcast (gpsimd, off the PE critical path)
                    for i, off in enumerate((0, HD)):
                        rrow = smp.tile([1, Q], F32, tag="rrow")
                        nc.vector.reciprocal(rrow[:], avs[i][HD:HD + 1, :])
                        lb = smp.tile([HD, Q], F32, tag="lb")
                        nc.gpsimd.partition_broadcast(lb[:], rrow[:])
                        nc.vector.tensor_mul(YT[ht][off:off + HD, :],
                                             avs[i][:HD, :], lb[:])

                # ---- output projection ---------------------------------
                for m in range(4):
                    osb = osbp.tile([P, DE], F32, tag="osb")
                    for ch in range(2):
                        pso = ps.tile([P, Q], F32, tag="ps")
                        nc.tensor.matmul(
                            pso[:], ones1[:], bp_row[:, ch * Q:(ch + 1) * Q],
                            start=True, stop=False)
                        for a in range(ET):
                            nc.tensor.matmul(
                                pso[:], YT[a][:, m * P:(m + 1) * P],
                                WP[a][:, ch * Q:(ch + 1) * Q],
                                start=False, stop=(a == ET - 1))
                        if ch == 0:
                            nc.scalar.copy(osb[:, :Q], pso[:])
                        else:
                            nc.vector.tensor_copy(osb[:, Q:], pso[:])
                    nc.sync.dma_start(out=out[m * P:(m + 1) * P, :],
                                      in_=osb[:])

    nc.compile()
    return nc


def get_nc():
    if "nc" not in _NC_CACHE:
        _NC_CACHE["nc"] = _build_nc()
    return _NC_CACHE["nc"]


def make_masks(qblocks):
    m = np.zeros((8, P, P), dtype=np.float32)
    for j in range(8):
        p = j // 2
        gq = P * qblocks[p] + np.arange(P)[None, :]
        gk = P * j + np.arange(P)[:, None]
        m[j] = (gk <= gq).astype(np.float32)
    return m


def shard_inputs(x_encoder, x_decoder, Wq, bq, Wk, bk, Wv, bv, Wp, bp):
    from ml_dtypes import bfloat16

    def bT(a):  # transpose + bf16, contiguous
        return np.ascontiguousarray(np.asarray(a, np.float32).T).astype(bfloat16)

    wqT, wkT, wvT, wpT = bT(Wq), bT(Wk), bT(Wv), bT(Wp)
    bq32 = np.ascontiguousarray(bq, dtype=np.float32)
    bk32 = np.ascontiguousarray(bk, dtype=np.float32)
    bv16 = np.asarray(bv, np.float32).astype(bfloat16)
    bp16 = np.asarray(bp, np.float32).astype(bfloat16)
    xeT = [bT(x_encoder[b]) for b in range(4)]
    msk = [make_masks(QB[h]).astype(bfloat16) for h in range(2)]

    in_maps = []
    for core in range(8):
        b, half = core // 2, core % 2
        xd = np.concatenate(
            [np.asarray(x_decoder[b][P * t:P * (t + 1)], np.float32)
             for t in QB[half]], 0)
        in_maps.append({
            "xdT": bT(xd),  # [DE, Q]
            "xeT": xeT[b],
            "wqT": wqT, "wkT": wkT, "wvT": wvT, "wpT": wpT,
            "bq": bq32, "bk": bk32, "bvb": bv16, "bpb": bp16,
            "masks": msk[half],
        })
    return in_maps


def assemble(results, B=4, T=1024):
    out = np.zeros((B, T, DE), dtype=np.float32)
    for core in range(8):
        b, half = core // 2, core % 2
        for p, t in enumerate(QB[half]):
            out[b, P * t:P * (t + 1)] = results[core]["out"][P * p:P * (p + 1)]
    return out


def kernel(**inputs):
    from concourse.bass_utils import run_bass_kernel_spmd
    nc = get_nc()
    in_maps = shard_inputs(**{k: np.asarray(v) for k, v in inputs.items()})
    res = run_bass_kernel_spmd(nc, in_maps, core_ids=list(range(8)))
    return assemble(res.results)


if __name__ == "__main__":
    nc = get_nc()
    print("built + compiled ok")


# revision 8
# speedup vs baseline: 2.1897x; 1.3084x over previous
"""Cross-attention (causal) Trainium2 kernel, 8-core SPMD, bf16 compute.

Sharding: core c -> batch c//2, decoder-row half c%2.
Half 0 owns 128-row q-blocks {0,3,4,7}, half 1 owns {1,2,5,6} of T_dec=1024
(balances causal work at 18 key-block units each); zero collectives, host
reassembles disjoint output rows.

Host-side layout prep (shard_inputs): X and W are pre-transposed to
emb-major bf16; the V bias is folded into the output-projection bias
(softmax weights sum to 1, so  out = yhat@WpT + (bv@WpT + bp));  masks and
biases are pre-arranged for contiguous DMA.  All matmuls are bf16 with
fp32 PSUM accumulation (end-to-end rel err ~2.4e-3, gate 2e-2).

Per-core kernel:
  QT/KT channel-major via W^T-panel matmuls; V token-major, augmented with
  a per-head ones column so softmax denominators come free as row 64 of
  the AV psum.  Attention runs per head-PAIR: heads 2t/2t+1 occupy PE
  row-groups 0-63/64-127 (concurrent S matmuls into the two banks of one
  2-bank st tile), and exp + causal-mask ops cover both heads in single
  strided instructions.  N_J trims each key block to the causally active
  q-column suffix; the tri/zero/ones mask window is always the first 128
  columns (host-supplied per-core masks).  The softmax tail (1/l) runs
  bcast -> reciprocal on 64 lanes -> multiply, spread over gpsimd+vector
  and decoupled from the PSUM banks by an early copy to SBUF.
"""

import numpy as np

P = 128
DE = 1024          # emb dim
Q = 512            # q rows per core
Q2 = 2 * Q
H = 16
HD = 64
ET = DE // P       # 8 e-tiles
# active q-cols per key-block j (shared max over both halves); the causal
# suffix starts at column 128*(j//2), so the mask window is always cols
# [0:128) of the active slice.
N_J = [512, 512, 384, 384, 256, 256, 128, 128]
QB = ([0, 3, 4, 7], [1, 2, 5, 6])                # q-block assignment per half

_NC_CACHE = {}


def _build_nc():
    import concourse.tile as tile
    from concourse import bacc, mybir

    F32 = mybir.dt.float32
    BF16 = mybir.dt.bfloat16
    AF = mybir.ActivationFunctionType

    nc = bacc.Bacc("TRN2", target_bir_lowering=False, debug=False)

    xdT = nc.dram_tensor("xdT", [DE, Q], BF16, kind="ExternalInput").ap()
    xeT = nc.dram_tensor("xeT", [DE, DE], BF16, kind="ExternalInput").ap()
    wqT = nc.dram_tensor("wqT", [DE, DE], BF16, kind="ExternalInput").ap()
    wkT = nc.dram_tensor("wkT", [DE, DE], BF16, kind="ExternalInput").ap()
    wvT = nc.dram_tensor("wvT", [DE, DE], BF16, kind="ExternalInput").ap()
    wpT = nc.dram_tensor("wpT", [DE, DE], BF16, kind="ExternalInput").ap()
    bqp = nc.dram_tensor("bqp", [P, ET], F32, kind="ExternalInput").ap()
    bkp = nc.dram_tensor("bkp", [P, ET], F32, kind="ExternalInput").ap()
    cb = nc.dram_tensor("cb", [DE], BF16, kind="ExternalInput").ap()
    masks2 = nc.dram_tensor("masks2", [P, 8, 2, P], BF16,
                            kind="ExternalInput").ap()
    out = nc.dram_tensor("out", [Q, DE], F32, kind="ExternalOutput").ap()

    with tile.TileContext(nc) as tc:
        with tc.tile_pool(name="persist", bufs=1) as pp, \
             tc.tile_pool(name="consts", bufs=1) as cp:
            # persistent activations + weight panels (bf16, emb-major)
            XdT = [pp.tile([P, Q], BF16, name=f"XdT{i}") for i in range(ET)]
            XeT = [pp.tile([P, DE], BF16, name=f"XeT{i}") for i in range(ET)]
            WQ = [pp.tile([P, DE], BF16, name=f"WQ{i}") for i in range(ET)]
            WK = [pp.tile([P, DE], BF16, name=f"WK{i}") for i in range(ET)]
            WV = [pp.tile([P, DE], BF16, name=f"WV{i}") for i in range(ET)]
            WP = [pp.tile([P, DE], BF16, name=f"WP{i}") for i in range(ET)]
            QT = [pp.tile([P, Q], BF16, name=f"QT{i}") for i in range(ET)]
            KT = [pp.tile([P, DE], BF16, name=f"KT{i}") for i in range(ET)]
            VA = [pp.tile([P, H * (HD + 1)], BF16, name=f"VA{i}")
                  for i in range(ET)]
            YT = [pp.tile([P, Q], BF16, name=f"YT{i}") for i in range(ET)]

            # DMA prefetch in consumption order: (XdT, WQ) -> (XeT, WK)
            # -> WV -> consts -> WP.
            for e in range(ET):
                nc.sync.dma_start(out=XdT[e], in_=xdT[e * P:(e + 1) * P, :])
                nc.sync.dma_start(out=WQ[e], in_=wqT[e * P:(e + 1) * P, :])
            for e in range(ET):
                nc.sync.dma_start(out=XeT[e], in_=xeT[e * P:(e + 1) * P, :])
                nc.sync.dma_start(out=WK[e], in_=wkT[e * P:(e + 1) * P, :])
            for e in range(ET):
                nc.sync.dma_start(out=WV[e], in_=wvT[e * P:(e + 1) * P, :])

            ones1 = cp.tile([1, P], BF16)
            nc.vector.memset(ones1, 1.0)
            ones16 = cp.tile([P, H], BF16)
            nc.vector.memset(ones16, 1.0)
            bq_sb = cp.tile([P, ET], F32)
            nc.gpsimd.dma_start(out=bq_sb, in_=bqp)
            bk_sb = cp.tile([P, ET], F32)
            nc.gpsimd.dma_start(out=bk_sb, in_=bkp)
            cb_row = cp.tile([1, DE], BF16)
            nc.gpsimd.dma_start(out=cb_row, in_=cb[None, :])
            masks_sb = cp.tile([P, 8, 2, P], BF16)
            nc.gpsimd.dma_start(out=masks_sb, in_=masks2)

            for e in range(ET):
                nc.sync.dma_start(out=WP[e], in_=wpT[e * P:(e + 1) * P, :])

            with tc.tile_pool(name="pt", bufs=3) as ptp, \
                 tc.tile_pool(name="ysb", bufs=2) as ysbp, \
                 tc.tile_pool(name="sm", bufs=3) as smp, \
                 tc.tile_pool(name="osb", bufs=2) as osbp:

                # ---- Q / K / V projections (3 PSUM banks) ---------------
                with tc.tile_pool(name="ps", bufs=3, space="PSUM") as ps:
                    for d in range(ET):
                        psq = ps.tile([P, Q], F32, tag="ps")
                        for e in range(ET):
                            nc.tensor.matmul(
                                psq[:], WQ[e][:, d * P:(d + 1) * P], XdT[e][:],
                                start=(e == 0), stop=(e == ET - 1))
                        nc.scalar.activation(QT[d][:], psq[:], AF.Identity,
                                             bias=bq_sb[:, d:d + 1])
                    for d in range(ET):
                        for ch in range(2):
                            psk = ps.tile([P, Q], F32, tag="ps")
                            for e in range(ET):
                                nc.tensor.matmul(
                                    psk[:], WK[e][:, d * P:(d + 1) * P],
                                    XeT[e][:, ch * Q:(ch + 1) * Q],
                                    start=(e == 0), stop=(e == ET - 1))
                            nc.scalar.activation(
                                KT[d][:, ch * Q:(ch + 1) * Q], psk[:],
                                AF.Identity, bias=bk_sb[:, d:d + 1])
                    # V token-major; bv is folded into the out-proj bias.
                    for kt in range(ET):
                        for ch in range(2):
                            psv = ps.tile([P, Q], F32, tag="ps")
                            for e in range(ET):
                                nc.tensor.matmul(
                                    psv[:], XeT[e][:, kt * P:(kt + 1) * P],
                                    WV[e][:, ch * Q:(ch + 1) * Q],
                                    start=(e == 0), stop=(e == ET - 1))
                            hbase = 8 * ch
                            dst = VA[kt][:, hbase * (HD + 1):
                                         (hbase + 8) * (HD + 1)]
                            dst = dst.rearrange(
                                "p (h x) -> p h x", h=8)[:, :, :HD]
                            src = psv.rearrange("p (h x) -> p h x", h=8)
                            nc.vector.tensor_copy(dst, src)
                        onesdst = VA[kt].rearrange(
                            "p (h x) -> p h x", x=HD + 1)[:, :, HD:HD + 1]
                        nc.vector.tensor_copy(
                            onesdst, ones16.rearrange("p (h x) -> p h x", x=1))

                # ---- attention, one head-pair per 2-bank st/av tile -----
                with tc.tile_pool(name="st", bufs=2, space="PSUM") as stp, \
                     tc.tile_pool(name="av", bufs=2, space="PSUM") as avp:

                    def emit_s(ht, j):
                        nj = N_J[j]
                        cs = P * (j // 2)
                        st = stp.tile([P, Q2], F32, tag="st")
                        for i, off in enumerate((0, HD)):
                            nc.tensor.matmul(
                                st[:, i * Q:i * Q + nj],
                                KT[ht][off:off + HD, j * P:(j + 1) * P],
                                QT[ht][off:off + HD, cs:cs + nj],
                                start=True, stop=True)
                        pt = ptp.tile([P, Q2], BF16, tag="pt")
                        st_v = st.rearrange("p (b c) -> p b c", b=2)[:, :, :nj]
                        pt_v = pt.rearrange("p (b c) -> p b c", b=2)[:, :, :nj]
                        nc.scalar.activation(pt_v, st_v, AF.Exp, scale=0.125)
                        pt_m = pt.rearrange("p (b c) -> p b c", b=2)[:, :, :P]
                        nc.vector.tensor_mul(pt_m, pt_m, masks_sb[:, j])
                        return pt

                    def emit_av(ht, j, pt, av):
                        nj = N_J[j]
                        cs = P * (j // 2)
                        for i, h in enumerate((2 * ht, 2 * ht + 1)):
                            nc.tensor.matmul(
                                av[:, i * Q + cs:i * Q + cs + nj],
                                VA[j][:, h * (HD + 1):(h + 1) * (HD + 1)],
                                pt[:, i * Q:i * Q + nj],
                                start=(j == 0), stop=(j == 7))

                    for ht in range(ET):
                        av = avp.tile([HD + 1, Q2], F32, name=f"av{ht}",
                                      tag="av")
                        pend = emit_s(ht, 0)
                        for j in range(8):
                            nxt = emit_s(ht, j + 1) if j < 7 else None
                            emit_av(ht, j, pend, av)
                            pend = nxt
                        # softmax tail: copy to SBUF (frees the psum banks),
                        # then bcast(l) -> 1/l on 64 lanes -> y*(1/l).
                        ysb = ysbp.tile([HD + 1, Q2], F32, name=f"ysb{ht}",
                                        tag="ysb")
                        nc.vector.tensor_copy(ysb[:], av[:])
                        # stage l on partition 0 (partition_broadcast reads
                        # the tile's partition 0 on hardware), via DMA ring
                        lrow = smp.tile([1, Q2], F32, tag="lrow")
                        nc.gpsimd.dma_start(out=lrow, in_=ysb[HD:HD + 1, :])
                        lb = smp.tile([HD, Q2], F32, tag="lb")
                        nc.gpsimd.partition_broadcast(lb[:], lrow[:])
                        rb = smp.tile([HD, Q2], F32, tag="rb")
                        nc.vector.reciprocal_approx_fast(out=rb[:], in_=lb[:])
                        nc.vector.tensor_mul(YT[ht][0:HD, :], ysb[:HD, :Q],
                                             rb[:, :Q])
                        nc.vector.tensor_mul(YT[ht][HD:P, :], ysb[:HD, Q:],
                                             rb[:, Q:])

                # ---- output projection ---------------------------------
                with tc.tile_pool(name="po", bufs=3, space="PSUM") as po:
                    for m in range(4):
                        osb = osbp.tile([P, DE], F32, tag="osb")
                        for ch in range(2):
                            pso = po.tile([P, Q], F32, tag="po")
                            nc.tensor.matmul(
                                pso[:], ones1[:],
                                cb_row[:, ch * Q:(ch + 1) * Q],
                                start=True, stop=False)
                            for a in range(ET):
                                nc.tensor.matmul(
                                    pso[:], YT[a][:, m * P:(m + 1) * P],
                                    WP[a][:, ch * Q:(ch + 1) * Q],
                                    start=False, stop=(a == ET - 1))
                            if ch == 0:
                                nc.scalar.copy(osb[:, :Q], pso[:])
                            else:
                                nc.vector.tensor_copy(osb[:, Q:], pso[:])
                        nc.sync.dma_start(out=out[m * P:(m + 1) * P, :],
                                          in_=osb[:])

    nc.compile()
    return nc


def get_nc():
    if "nc" not in _NC_CACHE:
        _NC_CACHE["nc"] = _build_nc()
    return _NC_CACHE["nc"]


def make_masks(qblocks):
    m = np.zeros((8, P, P), dtype=np.float32)
    for j in range(8):
        p = j // 2
        gq = P * qblocks[p] + np.arange(P)[None, :]
        gk = P * j + np.arange(P)[:, None]
        m[j] = (gk <= gq).astype(np.float32)
    return m


def shard_inputs(x_encoder, x_decoder, Wq, bq, Wk, bk, Wv, bv, Wp, bp):
    from ml_dtypes import bfloat16

    def bT(a):  # transpose + bf16, contiguous
        return np.ascontiguousarray(np.asarray(a, np.float32).T).astype(bfloat16)

    wqT, wkT, wvT, wpT = bT(Wq), bT(Wk), bT(Wv), bT(Wp)
    bqp = np.ascontiguousarray(
        np.asarray(bq, np.float32).reshape(ET, P).T)
    bkp = np.ascontiguousarray(
        np.asarray(bk, np.float32).reshape(ET, P).T)
    # bv rides through softmax (weights sum to 1): fold into out-proj bias
    cb = (np.asarray(bv, np.float32) @ np.asarray(Wp, np.float32).T
          + np.asarray(bp, np.float32)).astype(bfloat16)
    xeT = [bT(x_encoder[b]) for b in range(4)]

    msk = []
    for h in range(2):
        m = make_masks(QB[h]).astype(bfloat16)      # [8, P, P]
        m = np.transpose(m, (1, 0, 2))              # [P(r), 8(j), P(c)]
        m = np.stack([m, m], axis=2)                # [P, 8, 2, P]
        msk.append(np.ascontiguousarray(m))

    in_maps = []
    for core in range(8):
        b, half = core // 2, core % 2
        xd = np.concatenate(
            [np.asarray(x_decoder[b][P * t:P * (t + 1)], np.float32)
             for t in QB[half]], 0)
        in_maps.append({
            "xdT": bT(xd),  # [DE, Q]
            "xeT": xeT[b],
            "wqT": wqT, "wkT": wkT, "wvT": wvT, "wpT": wpT,
            "bqp": bqp, "bkp": bkp, "cb": cb,
            "masks2": msk[half],
        })
    return in_maps


def assemble(results, B=4, T=1024):
    out = np.zeros((B, T, DE), dtype=np.float32)
    for core in range(8):
        b, half = core // 2, core % 2
        for p, t in enumerate(QB[half]):
            out[b, P * t:P * (t + 1)] = results[core]["out"][P * p:P * (p + 1)]
    return out


def kernel(**inputs):
    from concourse.bass_utils import run_bass_kernel_spmd
    nc = get_nc()
    in_maps = shard_inputs(**{k: np.asarray(v) for k, v in inputs.items()})
    res = run_bass_kernel_spmd(nc, in_maps, core_ids=list(range(8)))
    return assemble(res.results)


if __name__ == "__main__":
    nc = get_nc()
    print("built + compiled ok")
